# revision 8
# baseline (speedup 1.0000x reference)
"""Trainium2 Bass kernel: 2-layer GAT (100k nodes, 1.6M edges) on 8 NeuronCores.

Strategy (dst-sharded graph parallel):
  - Nodes dst-sharded contiguously across 8 cores (12500 each), degree-sorted
    within each shard so 128-node tiles have near-uniform in-degree.
  - Per layer, each core computes a "message table" row per owned node:
    [h (128 f32) | a_src (2) | a_dst (2) | 1.0 | pad] = 768B rows,
    then an AllGather replicates the full 100352-row table to every core.
  - Edge phase is slot-major: for each 128-dst tile, slot k gathers each dst's
    k-th incoming edge's source row via indirect DMA (dst == partition), so
    attention softmax is dense per-partition math and aggregation is a PSUM
    accumulation of identity matmuls over scaled gathered rows.
  - Segment softmax uses exp(leaky_relu(logit)) without max-subtraction
    (logits are O(10), safe in fp32); divide by the summed denominator at the
    node level after aggregation.
"""

import sys

sys.path.insert(0, "/opt/trn_rl_repo")
sys.path.insert(0, "/root/.axon_site/_ro/trn_rl_repo")

import numpy as np

CORES = 8
TILE = 128
ROW = 136  # fp32 elements per table row (544 bytes)
AUXC = 128  # column where [asrc0, asrc1, adst0, adst1, one] begins
HID = 64
HEADS = 2
NEG_SLOPE = 0.2
import os as _os
_NUM_Q = int(_os.environ.get("GAT_NUM_SWDGE_QUEUES", "1"))

_RUNNER_CACHE = {}


# ----------------------------------------------------------------------------
# Host-side preprocessing
# ----------------------------------------------------------------------------

def _host_prep(x, edge_index):
    n_nodes = x.shape[0]
    shard = n_nodes // CORES
    ntiles = (shard + TILE - 1) // TILE
    shard_pad = ntiles * TILE

    src = np.asarray(edge_index[0], dtype=np.int64)
    dst = np.asarray(edge_index[1], dtype=np.int64)
    loops = np.arange(n_nodes, dtype=np.int64)
    src = np.concatenate([src, loops])
    dst = np.concatenate([dst, loops])

    owner = dst // shard

    per_core = []
    deg_by_pos_all = np.zeros((CORES, shard_pad), dtype=np.int64)
    pos_all = np.zeros(n_nodes, dtype=np.int64)  # permuted position of each node
    orders = []

    for c in range(CORES):
        m = owner == c
        s_c = src[m]
        d_loc = dst[m] - c * shard
        deg = np.bincount(d_loc, minlength=shard)
        deg_full = np.concatenate([deg, np.zeros(shard_pad - shard, dtype=deg.dtype)])
        order = np.argsort(deg_full, kind="stable")  # order[j] = local node at pos j
        pos = np.empty(shard_pad, dtype=np.int64)
        pos[order] = np.arange(shard_pad)
        orders.append(order)
        pos_all[c * shard:(c + 1) * shard] = pos[:shard]
        deg_by_pos_all[c] = deg_full[order]
        per_core.append((s_c, d_loc, pos))

    # common K schedule across cores (SPMD: one program)
    K_sched = []
    for t in range(ntiles):
        k = int(deg_by_pos_all[:, t * TILE:(t + 1) * TILE].max())
        K_sched.append(max(k, 1))
    tile_base = np.concatenate([[0], np.cumsum([TILE * k for k in K_sched])])
    S = int(tile_base[-1])

    # global table row of node s
    row_map = (owner_of := np.arange(n_nodes) // shard) * shard_pad + pos_all
    row_map = row_map.astype(np.int32)
    del owner_of

    idxs, masks, xTs = [], [], []
    K_arr = np.asarray(K_sched, dtype=np.int64)
    for c in range(CORES):
        s_c, d_loc, pos = per_core[c]
        p = pos[d_loc]
        ord_e = np.argsort(p, kind="stable")
        p_s = p[ord_e]
        s_s = s_c[ord_e]
        counts = np.bincount(p_s, minlength=shard_pad)
        cum = np.concatenate([[0], np.cumsum(counts)])
        k_slot = np.arange(len(p_s)) - cum[p_s]
        t_of = p_s // TILE
        d_of = p_s % TILE
        flat = tile_base[t_of] + d_of * K_arr[t_of] + k_slot

        idx_flat = np.zeros(S, dtype=np.int32)
        mask_flat = np.zeros(S, dtype=np.float32)
        idx_flat[flat] = row_map[s_s]
        mask_flat[flat] = 1.0
        # zero-degree (phantom) positions: enable slot 0 with row 0 so the
        # softmax denominator stays finite (their outputs are discarded).
        zp = np.nonzero(counts == 0)[0]
        if len(zp):
            zflat = tile_base[zp // TILE] + (zp % TILE) * K_arr[zp // TILE]
            mask_flat[zflat] = 1.0
        idxs.append(idx_flat)
        masks.append(mask_flat)

        x_pad = np.zeros((shard_pad, x.shape[1]), dtype=np.float32)
        x_pad[:shard] = x[c * shard:(c + 1) * shard]
        x_perm = x_pad[orders[c]]
        xTs.append(np.ascontiguousarray(x_perm.T))

    return {
        "shard": shard,
        "ntiles": ntiles,
        "shard_pad": shard_pad,
        "K_sched": K_sched,
        "S": S,
        "idxs": idxs,
        "masks": masks,
        "xTs": xTs,
        "orders": orders,
    }


def _lift_heads(a2x64):
    """[2, 64] per-head vectors -> [128, 2] block-diagonal lift."""
    out = np.zeros((2 * HID, HEADS), dtype=np.float32)
    for h in range(HEADS):
        out[h * HID:(h + 1) * HID, h] = a2x64[h]
    return out


def _make_weight_inputs(W1, as1, ad1, b1, W2, as2, ad2, b2, lin_w, lin_b):
    W1 = np.asarray(W1, np.float32)
    W2 = np.asarray(W2, np.float32)
    asad1 = np.concatenate([_lift_heads(np.asarray(as1, np.float32)),
                            _lift_heads(np.asarray(ad1, np.float32))], axis=1)  # [128,4]
    asad2_l = np.concatenate([_lift_heads(np.asarray(as2, np.float32)),
                              _lift_heads(np.asarray(ad2, np.float32))], axis=1)  # [128,4]
    wasad2 = np.ascontiguousarray(W2 @ asad2_l)  # [64, 4]
    b1r = np.tile(np.asarray(b1, np.float32)[None, :], (TILE, 1))  # [128, 64]
    b2r = np.tile(np.asarray(b2, np.float32)[None, :], (TILE, 1))
    linw = np.asarray(lin_w, np.float32).reshape(HID, 1)
    linbr = np.tile(np.asarray(lin_b, np.float32).reshape(1, 1), (TILE, 1))  # [128,1]
    return {
        "w1": np.ascontiguousarray(W1),
        "asad1": np.ascontiguousarray(asad1),
        "w2": np.ascontiguousarray(W2),
        "wasad2": wasad2,
        "b1r": np.ascontiguousarray(b1r),
        "b2r": np.ascontiguousarray(b2r),
        "linw": np.ascontiguousarray(linw),
        "linbr": np.ascontiguousarray(linbr),
    }


# ----------------------------------------------------------------------------
# Device program
# ----------------------------------------------------------------------------

def _build_program(K_sched, shard_pad, in_ch):
    import concourse.bass as bass
    import concourse.mybir as mybir
    from concourse import bacc
    from concourse.tile import TileContext
    from concourse.masks import make_identity

    f32 = mybir.dt.float32
    i32 = mybir.dt.int32
    ntiles = len(K_sched)
    S = TILE * int(np.sum(K_sched))
    tbl_rows = CORES * shard_pad
    tile_base = np.concatenate([[0], np.cumsum([TILE * k for k in K_sched])])

    nc = bacc.Bacc("TRN2", target_bir_lowering=False, debug=False,
                   num_devices=CORES, num_swdge_queues=_NUM_Q)

    xT = nc.dram_tensor("xT", [in_ch, shard_pad], f32, kind="ExternalInput")
    idxd = nc.dram_tensor("idx", [S], i32, kind="ExternalInput")
    maskd = nc.dram_tensor("mask", [S], f32, kind="ExternalInput")
    w1d = nc.dram_tensor("w1", [in_ch, 2 * HID], f32, kind="ExternalInput")
    asad1d = nc.dram_tensor("asad1", [2 * HID, 4], f32, kind="ExternalInput")
    w2d = nc.dram_tensor("w2", [HID, 2 * HID], f32, kind="ExternalInput")
    wasad2d = nc.dram_tensor("wasad2", [HID, 4], f32, kind="ExternalInput")
    b1rd = nc.dram_tensor("b1r", [TILE, HID], f32, kind="ExternalInput")
    b2rd = nc.dram_tensor("b2r", [TILE, HID], f32, kind="ExternalInput")
    linwd = nc.dram_tensor("linw", [HID, 1], f32, kind="ExternalInput")
    linbrd = nc.dram_tensor("linbr", [TILE, 1], f32, kind="ExternalInput")
    yd = nc.dram_tensor("y", [shard_pad, 1], f32, kind="ExternalOutput")

    with TileContext(nc) as tc:
        with (
            tc.tile_pool(name="const", bufs=1) as cpool,
            tc.tile_pool(name="work", bufs=4) as wpool,
            tc.tile_pool(name="gpool", bufs=5) as gpool,
            tc.tile_pool(name="psum", bufs=4, space="PSUM") as ppool,
            tc.tile_pool(name="dram", bufs=1, space="DRAM") as dpool,
        ):
            # ---- persistent tables in HBM
            t1s = dpool.tile([shard_pad, ROW], f32, tag="t1s")
            t1f = dpool.tile([tbl_rows, ROW], f32, addr_space="Shared", tag="t1f")
            t2s = dpool.tile([shard_pad, ROW], f32, tag="t2s")
            t2f = dpool.tile([tbl_rows, ROW], f32, addr_space="Shared", tag="t2f")

            # ---- constants
            ident = cpool.tile([TILE, TILE], f32, tag="ident")
            make_identity(nc, ident)
            w1_sb = cpool.tile([in_ch, 2 * HID], f32, tag="w1")
            nc.sync.dma_start(out=w1_sb, in_=w1d[:])
            asad1_sb = cpool.tile([2 * HID, 4], f32, tag="asad1")
            nc.sync.dma_start(out=asad1_sb, in_=asad1d[:])
            w2_sb = cpool.tile([HID, 2 * HID], f32, tag="w2")
            nc.sync.dma_start(out=w2_sb, in_=w2d[:])
            wasad2_sb = cpool.tile([HID, 4], f32, tag="wasad2")
            nc.sync.dma_start(out=wasad2_sb, in_=wasad2d[:])
            b1r_sb = cpool.tile([TILE, HID], f32, tag="b1r")
            nc.sync.dma_start(out=b1r_sb, in_=b1rd[:])
            b2r_sb = cpool.tile([TILE, HID], f32, tag="b2r")
            nc.sync.dma_start(out=b2r_sb, in_=b2rd[:])
            linw_sb = cpool.tile([HID, 1], f32, tag="linw")
            nc.sync.dma_start(out=linw_sb, in_=linwd[:])
            linbr_sb = cpool.tile([TILE, 1], f32, tag="linbr")
            nc.sync.dma_start(out=linbr_sb, in_=linbrd[:])
            xT_sb = cpool.tile([in_ch, shard_pad], f32, tag="xT")
            nc.sync.dma_start(out=xT_sb, in_=xT[:])
            aux1_sb = cpool.tile([TILE, ntiles * 4], f32, tag="aux1")
            aux2_sb = cpool.tile([TILE, ntiles * 4], f32, tag="aux2")

            # ---- phase A1: layer-1 table rows
            for t in range(ntiles):
                hT_ps = ppool.tile([TILE, TILE], f32, tag="mm_ps")
                nc.tensor.matmul(out=hT_ps[:], lhsT=w1_sb[:],
                                 rhs=xT_sb[:, t * TILE:(t + 1) * TILE],
                                 start=True, stop=True)
                hT_sb = wpool.tile([TILE, TILE], f32, tag="hT_sb")
                nc.vector.tensor_copy(out=hT_sb[:], in_=hT_ps[:])
                aux_ps = ppool.tile([TILE, 4], f32, tag="sm_ps", bufs=2)
                nc.tensor.matmul(out=aux_ps[:], lhsT=hT_sb[:], rhs=asad1_sb[:],
                                 start=True, stop=True)
                nc.vector.tensor_copy(out=aux1_sb[:, t * 4:(t + 1) * 4], in_=aux_ps[:])
                h_ps = ppool.tile([TILE, TILE], f32, tag="mm_ps")
                nc.tensor.transpose(out=h_ps[:], in_=hT_sb[:], identity=ident[:])
                row_sb = wpool.tile([TILE, ROW], f32, tag="row_sb")
                nc.vector.tensor_copy(out=row_sb[:, 0:TILE], in_=h_ps[:])
                nc.vector.tensor_copy(out=row_sb[:, AUXC:AUXC + 4],
                                      in_=aux1_sb[:, t * 4:(t + 1) * 4])
                nc.vector.memset(row_sb[:, AUXC + 4:AUXC + 5], 1.0)
                nc.sync.dma_start(out=t1s[t * TILE:(t + 1) * TILE, :], in_=row_sb[:])

            nc.gpsimd.collective_compute(
                "AllGather", mybir.AluOpType.bypass,
                replica_groups=[list(range(CORES))],
                ins=[t1s.opt()], outs=[t1f.opt()],
            )

            # ---- edge phase (shared for both layers)
            def edge_phase(t, tbl_full, aux_sb, br_sb):
                K = K_sched[t]
                base = int(tile_base[t])
                idx_sb = wpool.tile([TILE, K], i32, tag="idx_sb")
                nc.sync.dma_start(
                    out=idx_sb[:],
                    in_=idxd[base:base + TILE * K].rearrange("(p k) -> p k", k=K))
                mask_sb = wpool.tile([TILE, K], f32, tag="mask_sb")
                nc.sync.dma_start(
                    out=mask_sb[:],
                    in_=maskd[base:base + TILE * K].rearrange("(p k) -> p k", k=K))

                G = gpool.tile([TILE, K * ROW], f32, tag="G")
                for k in range(K):
                    inst = nc.gpsimd.indirect_dma_start(
                        out=G[:, k * ROW:(k + 1) * ROW], out_offset=None,
                        in_=tbl_full[:],
                        in_offset=bass.IndirectOffsetOnAxis(
                            ap=idx_sb[:, k:k + 1], axis=0))
                    if _NUM_Q > 1:
                        q = k % _NUM_Q
                        inst.queue = f"qPoolDynamic{q or ''}"
                G3 = G[:].rearrange("p (k r) -> p k r", r=ROW)

                # logits: t = a_src[src] + a_dst[dst]
                tS = wpool.tile([TILE, K * HEADS], f32, tag="tS")
                tS3 = tS[:].rearrange("p (k h) -> p k h", h=HEADS)
                adst = aux_sb[:, t * 4 + 2:t * 4 + 4]
                adst_b = adst.rearrange("p (o h) -> p o h", o=1).to_broadcast(
                    [TILE, K, HEADS])
                nc.vector.tensor_tensor(out=tS3, in0=G3[:, :, AUXC:AUXC + 2],
                                        in1=adst_b, op=mybir.AluOpType.add)
                # ex = exp(leaky_relu(t)) * mask
                tmpL = wpool.tile([TILE, K * HEADS], f32, tag="tmpL")
                nc.vector.tensor_scalar(out=tmpL[:], in0=tS[:],
                                        scalar1=NEG_SLOPE, scalar2=None,
                                        op0=mybir.AluOpType.mult)
                nc.vector.tensor_tensor(out=tS[:], in0=tS[:], in1=tmpL[:],
                                        op=mybir.AluOpType.max)
                nc.scalar.activation(out=tS[:], in_=tS[:],
                                     func=mybir.ActivationFunctionType.Exp)
                mask3 = mask_sb[:].rearrange("p (k o) -> p k o", o=1).to_broadcast(
                    [TILE, K, HEADS])
                nc.vector.tensor_tensor(out=tS3, in0=tS3, in1=mask3,
                                        op=mybir.AluOpType.mult)
                # denominators
                denom = wpool.tile([TILE, HEADS], f32, tag="denom")
                for h in range(HEADS):
                    nc.vector.tensor_reduce(
                        out=denom[:, h:h + 1],
                        in_=tS3[:, :, h:h + 1].rearrange("p k o -> p (k o)"),
                        op=mybir.AluOpType.add, axis=mybir.AxisListType.X)
                # scale gathered h by ex (per head, in place)
                for h in range(HEADS):
                    Gh = G3[:, :, h * HID:(h + 1) * HID]
                    exb = tS3[:, :, h:h + 1].to_broadcast([TILE, K, HID])
                    nc.vector.tensor_tensor(out=Gh, in0=Gh, in1=exb,
                                            op=mybir.AluOpType.mult)
                # aggregate: PSUM += I @ scaled rows
                out_ps = ppool.tile([TILE, 2 * HID], f32, tag="mm_ps")
                for k in range(K):
                    nc.tensor.matmul(out=out_ps[:], lhsT=ident[:],
                                     rhs=G[:, k * ROW:k * ROW + 2 * HID],
                                     start=(k == 0), stop=(k == K - 1))
                # mean over heads / denom + bias
                o_sb = wpool.tile([TILE, HID], f32, tag="o_sb")
                o2_sb = wpool.tile([TILE, HID], f32, tag="o2_sb")
                rden = wpool.tile([TILE, HEADS], f32, tag="rden")
                nc.vector.reciprocal(out=rden[:], in_=denom[:])
                nc.vector.tensor_scalar(out=o_sb[:], in0=out_ps[:, 0:HID],
                                        scalar1=rden[:, 0:1], scalar2=0.5,
                                        op0=mybir.AluOpType.mult,
                                        op1=mybir.AluOpType.mult)
                nc.vector.tensor_scalar(out=o2_sb[:], in0=out_ps[:, HID:2 * HID],
                                        scalar1=rden[:, 1:2], scalar2=0.5,
                                        op0=mybir.AluOpType.mult,
                                        op1=mybir.AluOpType.mult)
                nc.vector.tensor_tensor(out=o_sb[:], in0=o_sb[:], in1=o2_sb[:],
                                        op=mybir.AluOpType.add)
                nc.vector.tensor_tensor(out=o_sb[:], in0=o_sb[:], in1=br_sb[:],
                                        op=mybir.AluOpType.add)
                # ELU: max(x,0)-1 + exp(min(x,0))
                e0 = wpool.tile([TILE, HID], f32, tag="e0")
                nc.vector.tensor_scalar(out=e0[:], in0=o_sb[:], scalar1=0.0,
                                        scalar2=-1.0, op0=mybir.AluOpType.max,
                                        op1=mybir.AluOpType.add)
                e1 = wpool.tile([TILE, HID], f32, tag="e1")
                nc.vector.tensor_scalar(out=e1[:], in0=o_sb[:], scalar1=0.0,
                                        scalar2=None, op0=mybir.AluOpType.min)
                nc.scalar.activation(out=e1[:], in_=e1[:],
                                     func=mybir.ActivationFunctionType.Exp)
                helu = wpool.tile([TILE, HID], f32, tag="helu")
                nc.vector.tensor_tensor(out=helu[:], in0=e0[:], in1=e1[:],
                                        op=mybir.AluOpType.add)
                return helu

            # ---- B1 + A2 fused
            for t in range(ntiles):
                h2 = edge_phase(t, t1f, aux1_sb, b1r_sb)
                h2T_ps = ppool.tile([HID, TILE], f32, tag="mm_ps")
                nc.tensor.transpose(out=h2T_ps[:], in_=h2[:], identity=ident[:])
                h2T_sb = wpool.tile([HID, TILE], f32, tag="h2T_sb")
                nc.vector.tensor_copy(out=h2T_sb[:], in_=h2T_ps[:])
                hl2_ps = ppool.tile([TILE, 2 * HID], f32, tag="mm_ps")
                nc.tensor.matmul(out=hl2_ps[:], lhsT=h2T_sb[:], rhs=w2_sb[:],
                                 start=True, stop=True)
                aux2_ps = ppool.tile([TILE, 4], f32, tag="sm_ps", bufs=2)
                nc.tensor.matmul(out=aux2_ps[:], lhsT=h2T_sb[:], rhs=wasad2_sb[:],
                                 start=True, stop=True)
                nc.vector.tensor_copy(out=aux2_sb[:, t * 4:(t + 1) * 4],
                                      in_=aux2_ps[:])
                row2_sb = wpool.tile([TILE, ROW], f32, tag="row2_sb")
                nc.vector.tensor_copy(out=row2_sb[:, 0:TILE], in_=hl2_ps[:])
                nc.vector.tensor_copy(out=row2_sb[:, AUXC:AUXC + 4],
                                      in_=aux2_sb[:, t * 4:(t + 1) * 4])
                nc.vector.memset(row2_sb[:, AUXC + 4:AUXC + 5], 1.0)
                nc.sync.dma_start(out=t2s[t * TILE:(t + 1) * TILE, :],
                                  in_=row2_sb[:])

            nc.gpsimd.collective_compute(
                "AllGather", mybir.AluOpType.bypass,
                replica_groups=[list(range(CORES))],
                ins=[t2s.opt()], outs=[t2f.opt()],
            )

            # ---- B2 + final linear
            for t in range(ntiles):
                h3 = edge_phase(t, t2f, aux2_sb, b2r_sb)
                h3T_ps = ppool.tile([HID, TILE], f32, tag="mm_ps")
                nc.tensor.transpose(out=h3T_ps[:], in_=h3[:], identity=ident[:])
                h3T_sb = wpool.tile([HID, TILE], f32, tag="h3T_sb")
                nc.vector.tensor_copy(out=h3T_sb[:], in_=h3T_ps[:])
                y_ps = ppool.tile([TILE, 1], f32, tag="sm_ps", bufs=2)
                nc.tensor.matmul(out=y_ps[:], lhsT=h3T_sb[:], rhs=linw_sb[:],
                                 start=True, stop=True)
                y_sb = wpool.tile([TILE, 1], f32, tag="y_sb")
                nc.vector.tensor_tensor(out=y_sb[:], in0=y_ps[:], in1=linbr_sb[:],
                                        op=mybir.AluOpType.add)
                nc.sync.dma_start(out=yd[t * TILE:(t + 1) * TILE, :], in_=y_sb[:])

    nc.compile()
    return nc


# ----------------------------------------------------------------------------
# SPMD execution via PJRT (axon)
# ----------------------------------------------------------------------------

class _SpmdRunner:
    def __init__(self, nc, n_cores):
        import jax
        from jax.sharding import Mesh, PartitionSpec
        from jax.experimental.shard_map import shard_map
        import concourse.mybir as mybir
        from concourse.bass2jax import (_bass_exec_p, partition_id_tensor,
                                        install_neuronx_cc_hook)

        install_neuronx_cc_hook()
        self.jax = jax
        self.n_cores = n_cores
        partition_name = (nc.partition_id_tensor.name
                          if nc.partition_id_tensor else None)
        in_names, out_names, out_avals, zero_outs = [], [], [], []
        for alloc in nc.m.functions[0].allocations:
            if not isinstance(alloc, mybir.MemoryLocationSet):
                continue
            name = alloc.memorylocations[0].name
            if alloc.kind == "ExternalInput":
                if name != partition_name:
                    in_names.append(name)
            elif alloc.kind == "ExternalOutput":
                out_names.append(name)
                shape = tuple(alloc.tensor_shape)
                dtype = mybir.dt.np(alloc.dtype)
                out_avals.append(jax.core.ShapedArray(shape, dtype))
                zero_outs.append(np.zeros(shape, dtype))
        self.in_names, self.out_names = in_names, out_names
        self.out_avals, self.zero_outs = out_avals, zero_outs
        n_params = len(in_names)
        self.n_params = n_params
        all_in_names = list(in_names) + list(out_names)
        if partition_name is not None:
            all_in_names.append(partition_name)

        def _body(*args):
            operands = list(args)
            if partition_name is not None:
                operands.append(partition_id_tensor())
            outs = _bass_exec_p.bind(
                *operands,
                out_avals=tuple(out_avals),
                in_names=tuple(all_in_names),
                out_names=tuple(out_names),
                lowering_input_output_aliases=(),
                sim_require_finite=False,
                sim_require_nnan=False,
                nc=nc,
            )
            return tuple(outs)

        devices = jax.devices()[:n_cores]
        self.mesh = Mesh(np.asarray(devices), ("core",))
        in_specs = (PartitionSpec("core"),) * (n_params + len(out_names))
        out_specs = (PartitionSpec("core"),) * len(out_names)
        self.fn = jax.jit(
            shard_map(_body, mesh=self.mesh, in_specs=in_specs,
                      out_specs=out_specs, check_rep=False),
            keep_unused=True,
        )

    def prep(self, in_maps):
        from jax.sharding import NamedSharding, PartitionSpec
        per_core = [[np.asarray(m[name]) for name in self.in_names]
                    for m in in_maps]
        concat_in = [
            np.concatenate([per_core[c][i] for c in range(self.n_cores)], axis=0)
            for i in range(self.n_params)
        ]
        concat_zeros = [
            np.zeros((self.n_cores * z.shape[0], *z.shape[1:]), z.dtype)
            for z in self.zero_outs
        ]
        sh = NamedSharding(self.mesh, PartitionSpec("core"))
        self.args = [self.jax.device_put(a, sh) for a in (concat_in + concat_zeros)]
        return self

    def run(self):
        outs = self.fn(*self.args)
        self.jax.block_until_ready(outs)
        return [
            {name: np.asarray(outs[i]).reshape(
                self.n_cores, *self.out_avals[i].shape)[c]
             for i, name in enumerate(self.out_names)}
            for c in range(self.n_cores)
        ]


# ----------------------------------------------------------------------------
# Public entry point
# ----------------------------------------------------------------------------

def kernel(x, edge_index, W1, as1, ad1, b1, W2, as2, ad2, b2, lin_w, lin_b):
    x = np.asarray(x, np.float32)
    edge_index = np.asarray(edge_index)
    prep = _host_prep(x, edge_index)
    weights = _make_weight_inputs(W1, as1, ad1, b1, W2, as2, ad2, b2,
                                  lin_w, lin_b)

    key = (tuple(prep["K_sched"]), prep["shard_pad"], x.shape[1])
    if key not in _RUNNER_CACHE:
        nc = _build_program(prep["K_sched"], prep["shard_pad"], x.shape[1])
        _RUNNER_CACHE[key] = _SpmdRunner(nc, CORES)
    runner = _RUNNER_CACHE[key]

    in_maps = []
    for c in range(CORES):
        m = {"xT": prep["xTs"][c], "idx": prep["idxs"][c],
             "mask": prep["masks"][c]}
        m.update(weights)
        in_maps.append(m)
    runner.prep(in_maps)
    results = runner.run()

    shard = prep["shard"]
    y_full = np.zeros(x.shape[0], dtype=np.float32)
    for c in range(CORES):
        y_c = results[c]["y"][:, 0]
        order = prep["orders"][c]
        real = order < shard
        y_full[c * shard + order[real]] = y_c[real]
    return y_full


# revision 11
# speedup vs baseline: 2.0158x; 2.0158x over previous
"""Trainium2 Bass kernel: 2-layer GAT (100k nodes, 1.6M edges) on 8 NeuronCores.

Strategy (dst-sharded graph parallel):
  - Nodes dst-sharded contiguously across 8 cores (12500 each), degree-sorted
    within each shard so 128-node tiles have near-uniform in-degree.
  - Per layer, each core computes a "message table" row per owned node:
    [h (128 f32) | a_src (2) | a_dst (2) | 1.0 | pad] = 544B rows,
    then an AllGather replicates the full 100352-row table to every core.
  - Edge phase is slot-major: for each 128-dst tile, slot k gathers each dst's
    k-th incoming edge's source row via indirect DMA (dst == partition), so
    attention softmax is dense per-partition math and aggregation is a PSUM
    accumulation of identity matmuls over scaled gathered rows.
  - Segment softmax uses exp(leaky_relu(logit)) without max-subtraction
    (logits are O(10), safe in fp32); divide by the summed denominator at the
    node level after aggregation.
"""

import sys

sys.path.insert(0, "/opt/trn_rl_repo")
sys.path.insert(0, "/root/.axon_site/_ro/trn_rl_repo")

import numpy as np

CORES = 8
TILE = 128
ROW = 136  # fp32 elements per table row (544 bytes)
AUXC = 128  # column where [asrc0, asrc1, adst0, adst1, one] begins
HID = 64
HEADS = 2
NEG_SLOPE = 0.2
import os as _os
_NUM_Q = int(_os.environ.get("GAT_NUM_SWDGE_QUEUES", "1"))

_RUNNER_CACHE = {}


# ----------------------------------------------------------------------------
# Host-side preprocessing
# ----------------------------------------------------------------------------

def _host_prep(x, edge_index):
    n_nodes = x.shape[0]
    shard = n_nodes // CORES
    ntiles = (shard + TILE - 1) // TILE
    shard_pad = ntiles * TILE

    src = np.asarray(edge_index[0], dtype=np.int64)
    dst = np.asarray(edge_index[1], dtype=np.int64)
    loops = np.arange(n_nodes, dtype=np.int64)
    src = np.concatenate([src, loops])
    dst = np.concatenate([dst, loops])

    owner = dst // shard

    per_core = []
    deg_by_pos_all = np.zeros((CORES, shard_pad), dtype=np.int64)
    pos_all = np.zeros(n_nodes, dtype=np.int64)  # permuted position of each node
    orders = []

    for c in range(CORES):
        m = owner == c
        s_c = src[m]
        d_loc = dst[m] - c * shard
        deg = np.bincount(d_loc, minlength=shard)
        deg_full = np.concatenate([deg, np.zeros(shard_pad - shard, dtype=deg.dtype)])
        order = np.argsort(deg_full, kind="stable")  # order[j] = local node at pos j
        pos = np.empty(shard_pad, dtype=np.int64)
        pos[order] = np.arange(shard_pad)
        orders.append(order)
        pos_all[c * shard:(c + 1) * shard] = pos[:shard]
        deg_by_pos_all[c] = deg_full[order]
        per_core.append((s_c, d_loc, pos))

    # common K schedule across cores (SPMD: one program)
    K_sched = []
    for t in range(ntiles):
        k = int(deg_by_pos_all[:, t * TILE:(t + 1) * TILE].max())
        K_sched.append(max(k, 1))
    tile_base = np.concatenate([[0], np.cumsum([TILE * k for k in K_sched])])
    S = int(tile_base[-1])

    # global table row of node s
    row_map = (owner_of := np.arange(n_nodes) // shard) * shard_pad + pos_all
    row_map = row_map.astype(np.int32)
    del owner_of

    idxs, masks, xTs = [], [], []
    K_arr = np.asarray(K_sched, dtype=np.int64)
    for c in range(CORES):
        s_c, d_loc, pos = per_core[c]
        p = pos[d_loc]
        ord_e = np.argsort(p, kind="stable")
        p_s = p[ord_e]
        s_s = s_c[ord_e]
        counts = np.bincount(p_s, minlength=shard_pad)
        cum = np.concatenate([[0], np.cumsum(counts)])
        k_slot = np.arange(len(p_s)) - cum[p_s]
        t_of = p_s // TILE
        d_of = p_s % TILE
        flat = tile_base[t_of] + d_of * K_arr[t_of] + k_slot

        idx_flat = np.zeros(S, dtype=np.int32)
        mask_flat = np.zeros(S, dtype=np.float32)
        idx_flat[flat] = row_map[s_s]
        mask_flat[flat] = 1.0
        # zero-degree (phantom) positions: enable slot 0 with row 0 so the
        # softmax denominator stays finite (their outputs are discarded).
        zp = np.nonzero(counts == 0)[0]
        if len(zp):
            zflat = tile_base[zp // TILE] + (zp % TILE) * K_arr[zp // TILE]
            mask_flat[zflat] = 1.0
        idxs.append(idx_flat)
        masks.append(mask_flat)

        x_pad = np.zeros((shard_pad, x.shape[1]), dtype=np.float32)
        x_pad[:shard] = x[c * shard:(c + 1) * shard]
        x_perm = x_pad[orders[c]]
        xTs.append(np.ascontiguousarray(x_perm.T))

    return {
        "shard": shard,
        "ntiles": ntiles,
        "shard_pad": shard_pad,
        "K_sched": K_sched,
        "S": S,
        "idxs": idxs,
        "masks": masks,
        "xTs": xTs,
        "orders": orders,
    }


def _lift_heads(a2x64):
    """[2, 64] per-head vectors -> [128, 2] block-diagonal lift."""
    out = np.zeros((2 * HID, HEADS), dtype=np.float32)
    for h in range(HEADS):
        out[h * HID:(h + 1) * HID, h] = a2x64[h]
    return out


def _make_weight_inputs(W1, as1, ad1, b1, W2, as2, ad2, b2, lin_w, lin_b):
    W1 = np.asarray(W1, np.float32)
    W2 = np.asarray(W2, np.float32)
    asad1 = np.concatenate([_lift_heads(np.asarray(as1, np.float32)),
                            _lift_heads(np.asarray(ad1, np.float32))], axis=1)  # [128,4]
    asad2_l = np.concatenate([_lift_heads(np.asarray(as2, np.float32)),
                              _lift_heads(np.asarray(ad2, np.float32))], axis=1)  # [128,4]
    wasad2 = np.ascontiguousarray(W2 @ asad2_l)  # [64, 4]
    b1r = np.tile(np.asarray(b1, np.float32)[None, :], (TILE, 1))  # [128, 64]
    b2r = np.tile(np.asarray(b2, np.float32)[None, :], (TILE, 1))
    linw = np.asarray(lin_w, np.float32).reshape(HID, 1)
    linbr = np.tile(np.asarray(lin_b, np.float32).reshape(1, 1), (TILE, 1))  # [128,1]
    return {
        "w1": np.ascontiguousarray(W1),
        "asad1": np.ascontiguousarray(asad1),
        "w2": np.ascontiguousarray(W2),
        "wasad2": wasad2,
        "b1r": np.ascontiguousarray(b1r),
        "b2r": np.ascontiguousarray(b2r),
        "linw": np.ascontiguousarray(linw),
        "linbr": np.ascontiguousarray(linbr),
    }


# ----------------------------------------------------------------------------
# Device program
# ----------------------------------------------------------------------------

def _build_program(K_sched, shard_pad, in_ch):
    import concourse.bass as bass
    import concourse.mybir as mybir
    from concourse import bacc
    from concourse.tile import TileContext
    from concourse.masks import make_identity

    f32 = mybir.dt.float32
    i32 = mybir.dt.int32
    ntiles = len(K_sched)
    S = TILE * int(np.sum(K_sched))
    tbl_rows = CORES * shard_pad
    tile_base = np.concatenate([[0], np.cumsum([TILE * k for k in K_sched])])

    nc = bacc.Bacc("TRN2", target_bir_lowering=False, debug=False,
                   num_devices=CORES, num_swdge_queues=_NUM_Q)

    xT = nc.dram_tensor("xT", [in_ch, shard_pad], f32, kind="ExternalInput")
    idxd = nc.dram_tensor("idx", [S], i32, kind="ExternalInput")
    maskd = nc.dram_tensor("mask", [S], f32, kind="ExternalInput")
    w1d = nc.dram_tensor("w1", [in_ch, 2 * HID], f32, kind="ExternalInput")
    asad1d = nc.dram_tensor("asad1", [2 * HID, 4], f32, kind="ExternalInput")
    w2d = nc.dram_tensor("w2", [HID, 2 * HID], f32, kind="ExternalInput")
    wasad2d = nc.dram_tensor("wasad2", [HID, 4], f32, kind="ExternalInput")
    b1rd = nc.dram_tensor("b1r", [TILE, HID], f32, kind="ExternalInput")
    b2rd = nc.dram_tensor("b2r", [TILE, HID], f32, kind="ExternalInput")
    linwd = nc.dram_tensor("linw", [HID, 1], f32, kind="ExternalInput")
    linbrd = nc.dram_tensor("linbr", [TILE, 1], f32, kind="ExternalInput")
    yd = nc.dram_tensor("y", [shard_pad, 1], f32, kind="ExternalOutput")

    with TileContext(nc) as tc:
        with (
            tc.tile_pool(name="const", bufs=1) as cpool,
            tc.tile_pool(name="work", bufs=4) as wpool,
            tc.tile_pool(name="gpool", bufs=4) as gpool,
            tc.tile_pool(name="psum", bufs=4, space="PSUM") as ppool,
            tc.tile_pool(name="dram", bufs=1, space="DRAM") as dpool,
        ):
            # ---- persistent tables in HBM
            t1s = dpool.tile([shard_pad, ROW], f32, tag="t1s")
            t1f = dpool.tile([tbl_rows, ROW], f32, addr_space="Shared", tag="t1f")
            t2s = dpool.tile([shard_pad, ROW], f32, tag="t2s")
            t2f = dpool.tile([tbl_rows, ROW], f32, addr_space="Shared", tag="t2f")

            # ---- constants
            ident = cpool.tile([TILE, TILE], f32, tag="ident")
            make_identity(nc, ident)
            w1_sb = cpool.tile([in_ch, 2 * HID], f32, tag="w1")
            nc.sync.dma_start(out=w1_sb, in_=w1d[:])
            asad1_sb = cpool.tile([2 * HID, 4], f32, tag="asad1")
            nc.sync.dma_start(out=asad1_sb, in_=asad1d[:])
            w2_sb = cpool.tile([HID, 2 * HID], f32, tag="w2")
            nc.sync.dma_start(out=w2_sb, in_=w2d[:])
            wasad2_sb = cpool.tile([HID, 4], f32, tag="wasad2")
            nc.sync.dma_start(out=wasad2_sb, in_=wasad2d[:])
            b1r_sb = cpool.tile([TILE, HID], f32, tag="b1r")
            nc.sync.dma_start(out=b1r_sb, in_=b1rd[:])
            b2r_sb = cpool.tile([TILE, HID], f32, tag="b2r")
            nc.sync.dma_start(out=b2r_sb, in_=b2rd[:])
            linw_sb = cpool.tile([HID, 1], f32, tag="linw")
            nc.sync.dma_start(out=linw_sb, in_=linwd[:])
            linbr_sb = cpool.tile([TILE, 1], f32, tag="linbr")
            nc.sync.dma_start(out=linbr_sb, in_=linbrd[:])
            xT_sb = cpool.tile([in_ch, shard_pad], f32, tag="xT")
            nc.sync.dma_start(out=xT_sb, in_=xT[:])
            aux1_sb = cpool.tile([TILE, ntiles * 4], f32, tag="aux1")
            aux2_sb = cpool.tile([TILE, ntiles * 4], f32, tag="aux2")

            # ---- phase A1: layer-1 table rows
            for t in range(ntiles):
                hT_ps = ppool.tile([TILE, TILE], f32, tag="mm_ps")
                nc.tensor.matmul(out=hT_ps[:], lhsT=w1_sb[:],
                                 rhs=xT_sb[:, t * TILE:(t + 1) * TILE],
                                 start=True, stop=True)
                hT_sb = wpool.tile([TILE, TILE], f32, tag="hT_sb")
                nc.vector.tensor_copy(out=hT_sb[:], in_=hT_ps[:])
                aux_ps = ppool.tile([TILE, 4], f32, tag="sm_ps", bufs=2)
                nc.tensor.matmul(out=aux_ps[:], lhsT=hT_sb[:], rhs=asad1_sb[:],
                                 start=True, stop=True)
                nc.vector.tensor_copy(out=aux1_sb[:, t * 4:(t + 1) * 4], in_=aux_ps[:])
                h_ps = ppool.tile([TILE, TILE], f32, tag="mm_ps")
                nc.tensor.transpose(out=h_ps[:], in_=hT_sb[:], identity=ident[:])
                row_sb = wpool.tile([TILE, ROW], f32, tag="row_sb")
                nc.vector.tensor_copy(out=row_sb[:, 0:TILE], in_=h_ps[:])
                nc.vector.tensor_copy(out=row_sb[:, AUXC:AUXC + 4],
                                      in_=aux1_sb[:, t * 4:(t + 1) * 4])
                nc.vector.memset(row_sb[:, AUXC + 4:AUXC + 5], 1.0)
                nc.sync.dma_start(out=t1s[t * TILE:(t + 1) * TILE, :], in_=row_sb[:])

            nc.gpsimd.collective_compute(
                "AllGather", mybir.AluOpType.bypass,
                replica_groups=[list(range(CORES))],
                ins=[t1s.opt()], outs=[t1f.opt()],
            )

            # ---- edge phase (shared for both layers)
            def edge_phase(t, tbl_full, aux_sb, br_sb):
                K = K_sched[t]
                base = int(tile_base[t])
                idx_sb = wpool.tile([TILE, K], i32, tag="idx_sb")
                nc.sync.dma_start(
                    out=idx_sb[:],
                    in_=idxd[base:base + TILE * K].rearrange("(p k) -> p k", k=K))
                mask_sb = wpool.tile([TILE, K], f32, tag="mask_sb")
                nc.sync.dma_start(
                    out=mask_sb[:],
                    in_=maskd[base:base + TILE * K].rearrange("(p k) -> p k", k=K))

                G = gpool.tile([TILE, K * ROW], f32, tag="G")
                for k in range(K):
                    inst = nc.gpsimd.indirect_dma_start(
                        out=G[:, k * ROW:(k + 1) * ROW], out_offset=None,
                        in_=tbl_full[:],
                        in_offset=bass.IndirectOffsetOnAxis(
                            ap=idx_sb[:, k:k + 1], axis=0))
                    if _NUM_Q > 1:
                        q = k % _NUM_Q
                        inst.queue = f"qPoolDynamic{q or ''}"
                G3 = G[:].rearrange("p (k r) -> p k r", r=ROW)

                # logits: t = a_src[src] + a_dst[dst]
                tS = wpool.tile([TILE, K * HEADS], f32, tag="tS")
                tS3 = tS[:].rearrange("p (k h) -> p k h", h=HEADS)
                adst = aux_sb[:, t * 4 + 2:t * 4 + 4]
                adst_b = adst.rearrange("p (o h) -> p o h", o=1).to_broadcast(
                    [TILE, K, HEADS])
                nc.vector.tensor_tensor(out=tS3, in0=G3[:, :, AUXC:AUXC + 2],
                                        in1=adst_b, op=mybir.AluOpType.add)
                # ex = exp(leaky_relu(t)) * mask
                tmpL = wpool.tile([TILE, K * HEADS], f32, tag="tmpL")
                nc.vector.tensor_scalar(out=tmpL[:], in0=tS[:],
                                        scalar1=NEG_SLOPE, scalar2=None,
                                        op0=mybir.AluOpType.mult)
                nc.vector.tensor_tensor(out=tS[:], in0=tS[:], in1=tmpL[:],
                                        op=mybir.AluOpType.max)
                nc.scalar.activation(out=tS[:], in_=tS[:],
                                     func=mybir.ActivationFunctionType.Exp)
                mask3 = mask_sb[:].rearrange("p (k o) -> p k o", o=1).to_broadcast(
                    [TILE, K, HEADS])
                nc.vector.tensor_tensor(out=tS3, in0=tS3, in1=mask3,
                                        op=mybir.AluOpType.mult)
                # denominators
                denom = wpool.tile([TILE, HEADS], f32, tag="denom")
                for h in range(HEADS):
                    nc.vector.tensor_reduce(
                        out=denom[:, h:h + 1],
                        in_=tS3[:, :, h:h + 1].rearrange("p k o -> p (k o)"),
                        op=mybir.AluOpType.add, axis=mybir.AxisListType.X)
                # scale gathered h by ex (per head, in place)
                for h in range(HEADS):
                    Gh = G3[:, :, h * HID:(h + 1) * HID]
                    exb = tS3[:, :, h:h + 1].to_broadcast([TILE, K, HID])
                    nc.vector.tensor_tensor(out=Gh, in0=Gh, in1=exb,
                                            op=mybir.AluOpType.mult)
                # aggregate: PSUM += I @ scaled rows
                out_ps = ppool.tile([TILE, 2 * HID], f32, tag="mm_ps")
                for k in range(K):
                    nc.tensor.matmul(out=out_ps[:], lhsT=ident[:],
                                     rhs=G[:, k * ROW:k * ROW + 2 * HID],
                                     start=(k == 0), stop=(k == K - 1))
                # mean over heads / denom + bias
                o_sb = wpool.tile([TILE, HID], f32, tag="o_sb")
                o2_sb = wpool.tile([TILE, HID], f32, tag="o2_sb")
                rden = wpool.tile([TILE, HEADS], f32, tag="rden")
                nc.vector.reciprocal(out=rden[:], in_=denom[:])
                nc.vector.tensor_scalar(out=o_sb[:], in0=out_ps[:, 0:HID],
                                        scalar1=rden[:, 0:1], scalar2=0.5,
                                        op0=mybir.AluOpType.mult,
                                        op1=mybir.AluOpType.mult)
                nc.vector.tensor_scalar(out=o2_sb[:], in0=out_ps[:, HID:2 * HID],
                                        scalar1=rden[:, 1:2], scalar2=0.5,
                                        op0=mybir.AluOpType.mult,
                                        op1=mybir.AluOpType.mult)
                nc.vector.tensor_tensor(out=o_sb[:], in0=o_sb[:], in1=o2_sb[:],
                                        op=mybir.AluOpType.add)
                nc.vector.tensor_tensor(out=o_sb[:], in0=o_sb[:], in1=br_sb[:],
                                        op=mybir.AluOpType.add)
                # ELU: max(x,0)-1 + exp(min(x,0))
                e0 = wpool.tile([TILE, HID], f32, tag="e0")
                nc.vector.tensor_scalar(out=e0[:], in0=o_sb[:], scalar1=0.0,
                                        scalar2=-1.0, op0=mybir.AluOpType.max,
                                        op1=mybir.AluOpType.add)
                e1 = wpool.tile([TILE, HID], f32, tag="e1")
                nc.vector.tensor_scalar(out=e1[:], in0=o_sb[:], scalar1=0.0,
                                        scalar2=None, op0=mybir.AluOpType.min)
                nc.scalar.activation(out=e1[:], in_=e1[:],
                                     func=mybir.ActivationFunctionType.Exp)
                helu = wpool.tile([TILE, HID], f32, tag="helu")
                nc.vector.tensor_tensor(out=helu[:], in0=e0[:], in1=e1[:],
                                        op=mybir.AluOpType.add)
                return helu

            # ---- B1 + A2 fused
            for t in range(ntiles):
                h2 = edge_phase(t, t1f, aux1_sb, b1r_sb)
                h2T_ps = ppool.tile([HID, TILE], f32, tag="mm_ps")
                nc.tensor.transpose(out=h2T_ps[:], in_=h2[:], identity=ident[:])
                h2T_sb = wpool.tile([HID, TILE], f32, tag="h2T_sb")
                nc.vector.tensor_copy(out=h2T_sb[:], in_=h2T_ps[:])
                hl2_ps = ppool.tile([TILE, 2 * HID], f32, tag="mm_ps")
                nc.tensor.matmul(out=hl2_ps[:], lhsT=h2T_sb[:], rhs=w2_sb[:],
                                 start=True, stop=True)
                aux2_ps = ppool.tile([TILE, 4], f32, tag="sm_ps", bufs=2)
                nc.tensor.matmul(out=aux2_ps[:], lhsT=h2T_sb[:], rhs=wasad2_sb[:],
                                 start=True, stop=True)
                nc.vector.tensor_copy(out=aux2_sb[:, t * 4:(t + 1) * 4],
                                      in_=aux2_ps[:])
                row2_sb = wpool.tile([TILE, ROW], f32, tag="row2_sb")
                nc.vector.tensor_copy(out=row2_sb[:, 0:TILE], in_=hl2_ps[:])
                nc.vector.tensor_copy(out=row2_sb[:, AUXC:AUXC + 4],
                                      in_=aux2_sb[:, t * 4:(t + 1) * 4])
                nc.vector.memset(row2_sb[:, AUXC + 4:AUXC + 5], 1.0)
                nc.sync.dma_start(out=t2s[t * TILE:(t + 1) * TILE, :],
                                  in_=row2_sb[:])

            nc.gpsimd.collective_compute(
                "AllGather", mybir.AluOpType.bypass,
                replica_groups=[list(range(CORES))],
                ins=[t2s.opt()], outs=[t2f.opt()],
            )

            # ---- B2 + final linear
            for t in range(ntiles):
                h3 = edge_phase(t, t2f, aux2_sb, b2r_sb)
                h3T_ps = ppool.tile([HID, TILE], f32, tag="mm_ps")
                nc.tensor.transpose(out=h3T_ps[:], in_=h3[:], identity=ident[:])
                h3T_sb = wpool.tile([HID, TILE], f32, tag="h3T_sb")
                nc.vector.tensor_copy(out=h3T_sb[:], in_=h3T_ps[:])
                y_ps = ppool.tile([TILE, 1], f32, tag="sm_ps", bufs=2)
                nc.tensor.matmul(out=y_ps[:], lhsT=h3T_sb[:], rhs=linw_sb[:],
                                 start=True, stop=True)
                y_sb = wpool.tile([TILE, 1], f32, tag="y_sb")
                nc.vector.tensor_tensor(out=y_sb[:], in0=y_ps[:], in1=linbr_sb[:],
                                        op=mybir.AluOpType.add)
                nc.sync.dma_start(out=yd[t * TILE:(t + 1) * TILE, :], in_=y_sb[:])

    nc.compile()
    return nc


# ----------------------------------------------------------------------------
# SPMD execution via PJRT (axon)
# ----------------------------------------------------------------------------

class _SpmdRunner:
    def __init__(self, nc, n_cores):
        import jax
        from jax.sharding import Mesh, PartitionSpec
        from jax.experimental.shard_map import shard_map
        import concourse.mybir as mybir
        from concourse.bass2jax import (_bass_exec_p, partition_id_tensor,
                                        install_neuronx_cc_hook)

        install_neuronx_cc_hook()
        self.jax = jax
        self.n_cores = n_cores
        partition_name = (nc.partition_id_tensor.name
                          if nc.partition_id_tensor else None)
        in_names, out_names, out_avals, zero_outs = [], [], [], []
        for alloc in nc.m.functions[0].allocations:
            if not isinstance(alloc, mybir.MemoryLocationSet):
                continue
            name = alloc.memorylocations[0].name
            if alloc.kind == "ExternalInput":
                if name != partition_name:
                    in_names.append(name)
            elif alloc.kind == "ExternalOutput":
                out_names.append(name)
                shape = tuple(alloc.tensor_shape)
                dtype = mybir.dt.np(alloc.dtype)
                out_avals.append(jax.core.ShapedArray(shape, dtype))
                zero_outs.append(np.zeros(shape, dtype))
        self.in_names, self.out_names = in_names, out_names
        self.out_avals, self.zero_outs = out_avals, zero_outs
        n_params = len(in_names)
        self.n_params = n_params
        all_in_names = list(in_names) + list(out_names)
        if partition_name is not None:
            all_in_names.append(partition_name)

        def _body(*args):
            operands = list(args)
            if partition_name is not None:
                operands.append(partition_id_tensor())
            outs = _bass_exec_p.bind(
                *operands,
                out_avals=tuple(out_avals),
                in_names=tuple(all_in_names),
                out_names=tuple(out_names),
                lowering_input_output_aliases=(),
                sim_require_finite=False,
                sim_require_nnan=False,
                nc=nc,
            )
            return tuple(outs)

        devices = jax.devices()[:n_cores]
        self.mesh = Mesh(np.asarray(devices), ("core",))
        in_specs = (PartitionSpec("core"),) * (n_params + len(out_names))
        out_specs = (PartitionSpec("core"),) * len(out_names)
        self.fn = jax.jit(
            shard_map(_body, mesh=self.mesh, in_specs=in_specs,
                      out_specs=out_specs, check_rep=False),
            keep_unused=True,
        )

    def prep(self, in_maps):
        from jax.sharding import NamedSharding, PartitionSpec
        per_core = [[np.asarray(m[name]) for name in self.in_names]
                    for m in in_maps]
        concat_in = [
            np.concatenate([per_core[c][i] for c in range(self.n_cores)], axis=0)
            for i in range(self.n_params)
        ]
        concat_zeros = [
            np.zeros((self.n_cores * z.shape[0], *z.shape[1:]), z.dtype)
            for z in self.zero_outs
        ]
        sh = NamedSharding(self.mesh, PartitionSpec("core"))
        self.args = [self.jax.device_put(a, sh) for a in (concat_in + concat_zeros)]
        return self

    def run(self):
        outs = self.fn(*self.args)
        self.jax.block_until_ready(outs)
        return [
            {name: np.asarray(outs[i]).reshape(
                self.n_cores, *self.out_avals[i].shape)[c]
             for i, name in enumerate(self.out_names)}
            for c in range(self.n_cores)
        ]


# ----------------------------------------------------------------------------
# Public entry point
# ----------------------------------------------------------------------------

def kernel(x, edge_index, W1, as1, ad1, b1, W2, as2, ad2, b2, lin_w, lin_b):
    x = np.asarray(x, np.float32)
    edge_index = np.asarray(edge_index)
    prep = _host_prep(x, edge_index)
    weights = _make_weight_inputs(W1, as1, ad1, b1, W2, as2, ad2, b2,
                                  lin_w, lin_b)

    key = (tuple(prep["K_sched"]), prep["shard_pad"], x.shape[1])
    if key not in _RUNNER_CACHE:
        nc = _build_program(prep["K_sched"], prep["shard_pad"], x.shape[1])
        _RUNNER_CACHE[key] = _SpmdRunner(nc, CORES)
    runner = _RUNNER_CACHE[key]

    in_maps = []
    for c in range(CORES):
        m = {"xT": prep["xTs"][c], "idx": prep["idxs"][c],
             "mask": prep["masks"][c]}
        m.update(weights)
        in_maps.append(m)
    runner.prep(in_maps)
    results = runner.run()

    shard = prep["shard"]
    y_full = np.zeros(x.shape[0], dtype=np.float32)
    for c in range(CORES):
        y_c = results[c]["y"][:, 0]
        order = prep["orders"][c]
        real = order < shard
        y_full[c * shard + order[real]] = y_c[real]
    return y_full
